# revision 47
# baseline (speedup 1.0000x reference)
"""Trainium2 Bass kernel for nn_C4StandardTransformer (MoE-routed transformer step).

kernel(**inputs) takes FULL inputs (state [32768,16] + expert weights), shards
the batch across 8 NeuronCores (pure data parallel), runs an on-device
MoE-routed Bass kernel per core, and returns the full [32768,16] output.

Design (v4):
 - attention softmax over length-1 axis => w == 1; attn = Wov @ xn,
   Wov = Wo @ Wv per expert; Wq/Wk dead.
 - opcode is integral => top-hat gate is g0 = sigmoid(10)^2 own-expert only
   (neighbors ~4.5e-5, dropped). g0 folded into constants.
 - counting-sort routing on DVE + one PE matmul (strict-upper prefix);
   scatter/gather desc-gen is Q7-bound (~6.7ns/idx), so both are split
   across the 4 SWDGE queues (cpu pair q runs queue q) for 4x parallel
   generation; dummy 16-idx scatters at t=0 pay the ~7us IRAM library
   load off the critical path.
 - gather descriptors are prepared right after the scatters (before any Y
   write exists in program order) so gen overlaps phase 2; explicit ysem
   makes the per-queue triggers wait for the Y writes to land.
 - p-major token labeling: SBUF st[p][n] = DRAM token p*32+n, so input and
   output DMAs are 128 x 2KB contiguous descriptors (not 4096 x 64B).
 - supergroups of 8 experts processed in an 8-token-stacked [128,160] fp16
   layout (residual fused into the attn matmul via lhsT = [g0*I | g0*Wov^T]).
 - scalar-engine LUT swaps cost 1.28us, so phase 2 is split into pass A
   (stats; Copy/Square only), one grouped Sqrt batch, and pass B (Silu only).
"""
import sys
import numpy as np

for _p in ("/opt/trn_rl_repo", "/root/.axon_site/_ro/trn_rl_repo"):
    if _p not in sys.path:
        sys.path.append(_p)

E, D, DFF, OPCODE, EPS = 39, 16, 64, 6, 1e-5
Bc = 4096            # tokens per core
P = 128              # partitions
NCOL = Bc // P       # 32 free-dim token slots per partition
PADSZ = 160          # rank slots per expert per core
NE = 40              # padded expert count (8*5)
NSG = 5              # supergroups
NROW = PADSZ * NE    # sorted buffer rows
RWH = 128            # sorted-buffer row width in fp16 (256B stride)
G0 = float(1.0 / (1.0 + np.exp(-10.0))) ** 2
NQ = 4               # SWDGE queues (4 Q7 core pairs)
QI = Bc // NQ        # idxs per queue (1024)
QN = NCOL // NQ      # token cols per queue (8)
WARMUP = 104          # PE keep-alive matmuls across the scatter window
EARLY_PREP = True    # issue gather desc prep before phase 2 (overlap gen)
PRELOAD = False       # dummy scatters at t=0 to preload the Q7 IRAM library


def prep_consts(Wv, Wo, W1, b1, W2, b2):
    """Host-side constant packing. Returns dict name -> np.ndarray."""
    Wov = np.einsum('ejv,evd->ejd', Wo, Wv).astype(np.float32)  # attn = Wov @ xn

    def wslot(Warr, e, fill_shape):
        if 0 <= e < E:
            return Warr[e]
        return np.zeros(fill_shape, Warr.dtype)

    consts = {}
    consts["c_iota"] = np.arange(E, dtype=np.float16).reshape(1, E)
    lt = np.tril(np.ones((NCOL, NCOL), np.float16), -1)  # mask[n, n'] = n' < n
    consts["c_ltmask"] = lt.reshape(1, NCOL, NCOL)
    consts["c_uones"] = np.triu(np.ones((P, P), np.float16), 1)
    consts["c_id16"] = np.eye(P, dtype=np.float16)
    onesbd = np.zeros((P, P), np.float16)
    for t in range(8):
        onesbd[t*16:(t+1)*16, t*16:(t+1)*16] = np.float16(1.0 / 16.0)
    consts["c_onesbd"] = onesbd
    esel = np.zeros((8, P, P), np.float16)
    for r in range(8):
        for m in range(P):
            esel[r, 16 * r + (m % 16), m] = 1.0
    consts["c_esel"] = np.ascontiguousarray(esel.transpose(1, 0, 2))  # [P,8,P]

    # fused x1 weights: psA[16j+d, c] = g0*(state_d + sum_v Wov[e][d,v] xn_v)
    wI = np.zeros((NSG, 2, P, 64), np.float16)
    b2s = np.zeros((NSG, P, 1), np.float32)
    wB = np.zeros((NSG, 4, P, P), np.float16)
    b1s = np.zeros((NSG, 4, P, 1), np.float32)
    wC = np.zeros((NSG, 4, P, 32), np.float16)
    for s in range(NSG):
        for t in range(8):
            e = 8 * s + t
            b2s[s, t*16:(t+1)*16, 0] = G0 * wslot(b2, e, (D,))
            h, j = t // 4, t % 4
            wv = wslot(Wov, e, (D, D))  # [d_out, v_in]
            for dd in range(D):
                wI[s, h, 32*j + dd, 16*j + dd] = np.float16(G0)
            wI[s, h, 32*j+16:32*j+32, 16*j:16*j+16] = \
                (G0 * wv.T).astype(np.float16)        # [v, d_out]
        for i in range(4):
            for tt in range(2):
                e = 8 * s + 2 * i + tt
                t = 2 * i + tt
                w1 = wslot(W1, e, (DFF, D))           # h[f] = sum_d w1[f,d] xn2[d]
                wB[s, i, t*16:(t+1)*16, tt*64:(tt+1)*64] = w1.T.astype(np.float16)
                b1s[s, i, tt*64:(tt+1)*64, 0] = wslot(b1, e, (DFF,))
                w2 = wslot(W2, e, (D, DFF))           # ffn[d] = sum_f w2[d,f] h[f]
                wC[s, i, tt*64:(tt+1)*64, tt*16:(tt+1)*16] = \
                    (G0 * w2.T).astype(np.float16)
    consts["c_wI"] = np.ascontiguousarray(wI.transpose(2, 0, 1, 3))      # [P,5,2,64]
    consts["c_b2s"] = np.ascontiguousarray(b2s.transpose(1, 0, 2))       # [P,5,1]
    consts["c_wB"] = np.ascontiguousarray(wB.transpose(2, 0, 1, 3))      # [P,5,4,128]
    consts["c_b1s"] = np.ascontiguousarray(b1s.transpose(2, 0, 1, 3))    # [P,5,4,1]
    consts["c_wC"] = np.ascontiguousarray(wC.transpose(2, 0, 1, 3))      # [P,5,4,32]
    return consts


def build_kernel(stop_after=None, debug=False, use_silu=True):
    """stop_after in (None, 'route', 'scatter', 'compute'). use_silu=False
    falls back to Sigmoid+mult (CoreSim has no Silu)."""
    import concourse.bass as bass
    import concourse.bacc as bacc
    import concourse.tile as tile
    from concourse import mybir
    from contextlib import ExitStack

    f32, f16, i16 = mybir.dt.float32, mybir.dt.float16, mybir.dt.int16
    AX = mybir.AxisListType.X
    OP = mybir.AluOpType
    ACTF = mybir.ActivationFunctionType

    nc = bacc.Bacc(None, target_bir_lowering=False, num_swdge_queues=NQ)

    state = nc.declare_dram_parameter("state", [Bc, D], f32, isOutput=False)
    out = nc.declare_dram_parameter("out", [Bc, D], f16, isOutput=True)

    cshape = {
        "c_iota": ([1, E], f16), "c_ltmask": ([1, NCOL, NCOL], f16),
        "c_uones": ([P, P], f16), "c_esel": ([P, 8, P], f16),
        "c_id16": ([P, P], f16), "c_onesbd": ([P, P], f16),
        "c_wI": ([P, NSG, 2, 64], f16), "c_b2s": ([P, NSG, 1], f32),
        "c_wB": ([P, NSG, 4, P], f16), "c_b1s": ([P, NSG, 4, 1], f32),
        "c_wC": ([P, NSG, 4, 32], f16),
    }
    cparams = {n: nc.declare_dram_parameter(n, list(sh), dt, isOutput=False)
               for n, (sh, dt) in cshape.items()}

    if debug:
        Y = nc.declare_dram_parameter("Y", [NE * 256, RWH], f16, isOutput=True)
    else:
        Y = nc.dram_tensor("Y", [NE * 256, RWH], f16)
    TRASH = nc.dram_tensor("TRASH", [16, RWH], f16)

    run2 = stop_after not in ("route", "scatter")
    run3 = run2 and stop_after != "compute"

    with tile.TileContext(nc) as tc, ExitStack() as ctx:
        cpool = ctx.enter_context(tc.tile_pool(name="consts", bufs=1))
        ppool = ctx.enter_context(tc.tile_pool(name="p1", bufs=1))
        gpool = ctx.enter_context(tc.tile_pool(name="p2", bufs=2))
        psp = ctx.enter_context(tc.tile_pool(name="psp", bufs=1, space="PSUM"))

        # ---- SWDGE ucode preload: 4 dummy 16-idx scatters (one per queue)
        # pay the ~7us extended-inst IRAM load at t=0, off the critical path,
        # and verify all 4 queue core-pairs are live. ----
        if PRELOAD:
            zidx = cpool.tile([P, 16], i16, tag="zidx")
            nc.vector.memset(zidx[:], 0)
            zpay = cpool.tile([P, 1, 2 * D], f16, tag="zpay")
            nc.vector.memset(zpay[:], 0.0)
            for q in range(NQ):
                nc.gpsimd.dma_scatter_add(
                    TRASH[:, 0:2*D], zpay[:], zidx[:, q:q+1], 16, 16, 2 * D,
                    elem_step=RWH, single_packet=False, queue_num=q)

        # ---- phase 1 input first (gates everything); p-major token order:
        # st[p][n] = DRAM token p*NCOL+n -> 128 contiguous 2KB descriptors ----
        st = ppool.tile([P, NCOL, D], f32, tag="st")
        nc.sync.dma_start(out=st[:], in_=state.rearrange("(p n) d -> p n d", p=P))

        # ---- constants into SBUF (alternate queues, overlap phase 1);
        # routing-critical consts go on sync so scalar is free for opvh ----
        ct = {}
        _crit = {"c_iota", "c_ltmask", "c_uones", "c_esel", "c_id16"}
        for ci, (n, (sh, dt)) in enumerate(cshape.items()):
            eng = nc.sync if n in _crit else nc.scalar
            if sh[0] == 1:
                rsh = [P] + list(sh[1:])
                t = cpool.tile(rsh, dt, tag=n)
                eng.dma_start(out=t[:], in_=cparams[n][:].to_broadcast(rsh))
            else:
                t = cpool.tile(sh, dt, tag=n)
                eng.dma_start(out=t[:], in_=cparams[n][:])
            ct[n] = t
        epsb = cpool.tile([P, 1], f32, tag="epsb")
        nc.vector.memset(epsb[:], EPS)

        # ---- sorted-token SBUF accumulators (parity-split scatter dst):
        # bufA holds ranks 0-127 (partition = rank), bufB ranks 128-159 on
        # partitions 0-31; free dim = expert column * 32 payload halves.
        # CCE-add needs them zeroed; memsets are engine-synchronous so the
        # scatter (ordered after by Tile WAW) adds onto zeros. ----
        ssems = [nc.alloc_semaphore(f"ssem{q}") for q in range(NQ)]
        bufA = ppool.tile([P, NE, 2 * D], f16, tag="bufA")
        bufB = ppool.tile([P, NE, 2 * D], f16, tag="bufB")
        nc.vector.memset(bufA[:], 0.0)
        nc.vector.memset(bufB[:], 0.0)
        zb = cpool.tile([P, 512], f16, tag="zb")
        nc.vector.memset(zb[:], 0.0)

        # ---- routing ----
        opvh = ppool.tile([P, NCOL], f16, tag="opvh")
        nc.scalar.copy(out=opvh[:], in_=st[:, :, OPCODE])
        eq39T = ppool.tile([P, E, NCOL], f16, tag="eq39T")
        nc.vector.tensor_tensor(
            out=eq39T[:],
            in0=opvh[:].rearrange("p (u n) -> p u n", u=1).to_broadcast([P, E, NCOL]),
            in1=ct["c_iota"][:].rearrange("p (e u) -> p e u", u=1).to_broadcast([P, E, NCOL]),
            op=OP.is_equal)
        rowcnth = ppool.tile([P, E], f16, tag="rowcnth")
        with nc.allow_low_precision(reason="counts <= 160 are fp16-exact"):
            nc.vector.tensor_reduce(out=rowcnth[:], in_=eq39T[:], axis=AX,
                                    op=OP.add)
        pidx = psp.tile([P, 512], f32, tag="pidx")
        nc.tensor.matmul(pidx[:, 0:E], ct["c_uones"][:], rowcnth[:], start=True,
                         stop=True)
        C1h = ppool.tile([P, E], f16, tag="C1h")
        nc.scalar.copy(out=C1h[:], in_=pidx[:, 0:E])
        eq39 = ppool.tile([P, NCOL, E], f16, tag="eq39")
        nc.vector.tensor_tensor(
            out=eq39[:],
            in0=opvh[:].rearrange("p (n u) -> p n u", u=1).to_broadcast([P, NCOL, E]),
            in1=ct["c_iota"][:].rearrange("p (u e) -> p u e", u=1).to_broadcast([P, NCOL, E]),
            op=OP.is_equal)
        # rank = sum_e onehot*C1h + sum_{n'<n} same-opcode: write both masked
        # products into one concatenated [E | NCOL] free axis; one reduce
        # yields the rank directly (saves a reduce + an add of serial DVE).
        M = ppool.tile([P, NCOL, E + NCOL], f16, tag="Mcat")
        nc.vector.tensor_tensor(
            out=M[:, :, 0:E], in0=eq39[:],
            in1=C1h[:].rearrange("p (u e) -> p u e", u=1).to_broadcast([P, NCOL, E]),
            op=OP.mult)
        nc.vector.tensor_tensor(
            out=M[:, :, E:E+NCOL],
            in0=opvh[:].rearrange("p (n u) -> p n u", u=1).to_broadcast([P, NCOL, NCOL]),
            in1=opvh[:].rearrange("p (u n) -> p u n", u=1).to_broadcast([P, NCOL, NCOL]),
            op=OP.is_equal)
        nc.vector.tensor_tensor(out=M[:, :, E:E+NCOL], in0=M[:, :, E:E+NCOL],
                                in1=ct["c_ltmask"][:], op=OP.mult)
        rnk = ppool.tile([P, NCOL], f16, tag="rnk")
        with nc.allow_low_precision(reason="counts <= 160 are fp16-exact"):
            nc.vector.tensor_reduce(out=rnk[:], in_=M[:], axis=AX, op=OP.add)

        # ---- idx relayout via PE: pidx[16k+q, 32r+n] = {rnk|opv}[16r+q, n]
        # (dst = 40*rank + e assembled below; replication across the 8
        # 16-partition stripes comes from the replicated selector) ----
        for r in range(8):
            nc.tensor.matmul(pidx[:, 32*r:32*(r+1)], ct["c_esel"][:, r, :],
                             rnk[:], start=True, stop=True)
            nc.tensor.matmul(pidx[:, 256+32*r:256+32*(r+1)],
                             ct["c_esel"][:, r, :], opvh[:], start=True,
                             stop=True)
        pe2c = ppool.tile([P, 256], f16, tag="pe2c")
        nc.scalar.copy(out=pe2c[:], in_=pidx[:, 256:512])
        t1 = ppool.tile([P, 256], f32, tag="t1")
        nc.vector.tensor_scalar(out=t1[:], in0=pe2c[:],
                                scalar1=256.0, scalar2=None, op0=OP.mult)
        nc.vector.tensor_tensor(out=t1[:], in0=t1[:], in1=pidx[:, 0:256],
                                op=OP.add)
        idx16 = ppool.tile([P, Bc // 16], i16, tag="idx16")
        nc.vector.tensor_copy(
            out=idx16[:].rearrange("p (n r) -> p n r", r=8),
            in_=t1[:].rearrange("p (r n) -> p n r", r=8))

        # ---- LN1 -> payload [state | xn] fp16 ----
        xnst = ppool.tile([P, NCOL, 2 * D], f16, tag="xnst")
        nc.scalar.copy(out=xnst[:, :, 0:D], in_=st[:])
        mh = ppool.tile([P, NCOL], f32, tag="mh")
        nc.vector.tensor_reduce(out=mh[:], in_=st[:], axis=AX, op=OP.add)
        nc.vector.tensor_scalar(out=mh[:], in0=mh[:], scalar1=1.0 / D,
                                scalar2=None, op0=OP.mult)
        sqt = ppool.tile([P, NCOL, D], f32, tag="sqt")
        nc.scalar.square(out=sqt[:], in_=st[:])
        qh = ppool.tile([P, NCOL], f32, tag="qh")
        nc.vector.tensor_reduce(out=qh[:], in_=sqt[:], axis=AX, op=OP.add)
        msq = ppool.tile([P, NCOL], f32, tag="msq")
        nc.vector.tensor_tensor(out=msq[:], in0=mh[:], in1=mh[:], op=OP.mult)
        nc.vector.tensor_scalar(out=qh[:], in0=qh[:], scalar1=1.0 / D,
                                scalar2=None, op0=OP.mult)
        nc.vector.tensor_tensor(out=qh[:], in0=qh[:], in1=msq[:], op=OP.subtract)
        sd = ppool.tile([P, NCOL], f32, tag="sd")
        nc.scalar.activation(out=sd[:], in_=qh[:], func=ACTF.Sqrt,
                             bias=epsb[:], scale=1.0)
        rs = ppool.tile([P, NCOL], f32, tag="rs")
        nc.vector.reciprocal_approx_fast(out=rs[:], in_=sd[:])
        xc = ppool.tile([P, NCOL, D], f32, tag="xc")
        nc.vector.tensor_tensor(
            out=xc[:], in0=st[:],
            in1=mh[:].rearrange("p (n u) -> p n u", u=1).to_broadcast([P, NCOL, D]),
            op=OP.subtract)
        nc.vector.tensor_tensor(
            out=xnst[:, :, D:2*D], in0=xc[:],
            in1=rs[:].rearrange("p (n u) -> p n u", u=1).to_broadcast([P, NCOL, D]),
            op=OP.mult)

        # ---- scatter, split across the 4 SWDGE queues: queue q moves token
        # cols [8q, 8q+8) (idx positions [1024q, 1024q+1024)); desc-gen runs
        # on Q7 core pair q, so all 4 generate concurrently. ----
        for q in range(NQ):
            nc.gpsimd.dma_scatter_add(
                bufA[:], xnst[:, QN*q:QN*(q+1), :],
                idx16[:, (QI//16)*q:(QI//16)*(q+1)], QI, QI, 2 * D,
                single_packet=True, queue_num=q,
                sbuf_tokens_per_rank=P, parity_reg=0,
                out_ap_other=bufB[:]).then_inc(ssems[q], 16)

        # ---- gather desc prep, also split across queues, issued BEFORE any
        # Y write exists in program order (so Tile attaches no Y-write wait);
        # core pair q runs it right after its scatter gen, hidden under the
        # scatter drain + phase 2. Triggered after ysem confirms Y landed. ----
        tc.no_sync_barrier()
        gsems = [nc.alloc_semaphore(f"gsem{q}") for q in range(NQ)]
        yg = ppool.tile([P, NCOL, RWH], f16, tag="yg")

        if not run2:
            nc.sync.dma_start(out=out.rearrange("(p n) d -> p n d", p=P),
                              in_=xnst[:, :, 0:D])

        # ---- PE warm-up chain: keeps the HAM clock at 2.4GHz through the
        # scatter window (PE otherwise idles and re-throttles).
        # WAW-chained on one psum bank; gated after the relayout reads. ----
        if run2:
            for _w in range(WARMUP):
                nc.tensor.matmul(pidx[:, 0:512], ct["c_id16"][:],
                                 zb[:, 0:512], start=True, stop=True)

        # ---- phase 2: supergroups ----
        Yv = Y.rearrange("(e c) w -> c e w", c=256)
        if run2:
            vvall = ppool.tile([P, NSG, PADSZ], f32, tag="vvall")
            x1call = ppool.tile([P, NSG, PADSZ], f16, tag="x1call")
            x1pball = ppool.tile([P, NSG, PADSZ], f32, tag="x1pball")
            sdall = ppool.tile([P, NSG, PADSZ], f32, tag="sdall")
            rsall = ppool.tile([P, NSG, PADSZ], f32, tag="rsall")
            xn2all = ppool.tile([P, NSG, PADSZ], f16, tag="xn2all")
        if run2:
            for q in range(NQ):
                nc.tensor.wait_ge(ssems[q], 16)
        # pass A: stats (ACT funcs: Copy/Square only)
        for s in range(NSG) if run2 else []:
            xh = [gpool.tile([P, PADSZ], f16, tag=f"xh{h}", name=f"xh{h}")
                  for h in range(2)]
            for h in range(2):
                eb = 8 * s + 4 * h
                pt = psp.tile([P, PADSZ], f16, tag="ptt")
                nc.tensor.transpose(
                    pt[:, 0:P], bufA[:, eb:eb+4, :].rearrange("p e v -> p (e v)"),
                    ct["c_id16"][:])
                nc.tensor.transpose(
                    pt[:, P:PADSZ],
                    bufB[0:32, eb:eb+4, :].rearrange("p e v -> p (e v)"),
                    ct["c_id16"][0:32, 0:32])
                nc.scalar.copy(out=xh[h][:], in_=pt[:])
            psA = psp.tile([P, PADSZ], f32, tag="psA", bufs=2)
            for h in range(2):
                nc.tensor.matmul(psA[64*h:64*(h+1), :], ct["c_wI"][:, s, h, :],
                                 xh[h][:], start=True, stop=True)
            x1sq = gpool.tile([P, 2 * PADSZ], f16, tag="x1sq")
            nc.vector.tensor_copy(out=x1sq[:, 0:PADSZ], in_=psA[:])
            nc.vector.tensor_tensor(out=x1sq[:, PADSZ:2*PADSZ],
                                    in0=x1sq[:, 0:PADSZ], in1=psA[:],
                                    op=OP.mult)
            psS = psp.tile([P, 2 * PADSZ], f32, tag="psS", bufs=2)
            nc.tensor.matmul(psS[:], ct["c_onesbd"][:], x1sq[:], start=True,
                             stop=True)
            mc = gpool.tile([P, PADSZ], f32, tag="mc")
            nc.scalar.copy(out=mc[:], in_=psS[:, 0:PADSZ])
            msq2 = gpool.tile([P, PADSZ], f32, tag="msq2")
            nc.vector.tensor_tensor(out=msq2[:], in0=mc[:], in1=mc[:], op=OP.mult)
            nc.vector.tensor_tensor(out=vvall[:, s, :],
                                    in0=psS[:, PADSZ:2*PADSZ],
                                    in1=msq2[:], op=OP.subtract)
            nc.vector.tensor_tensor(out=x1call[:, s, :], in0=psA[:], in1=mc[:],
                                    op=OP.subtract)
            nc.vector.tensor_scalar(out=x1pball[:, s, :], in0=psA[:],
                                    scalar1=ct["c_b2s"][:, s, :],
                                    scalar2=None, op0=OP.add)
            # per-sg LN2 tail: Copy/Square are tableless, so the Sqrt table
            # stays resident through all of pass A (one load total).
            nc.scalar.activation(out=sdall[:, s, :], in_=vvall[:, s, :],
                                 func=ACTF.Sqrt, bias=epsb[:], scale=1.0)
            nc.vector.reciprocal_approx_fast(out=rsall[:, s, :],
                                             in_=sdall[:, s, :])
            nc.vector.tensor_tensor(out=xn2all[:, s, :],
                                    in0=x1call[:, s, :],
                                    in1=rsall[:, s, :], op=OP.mult)
        # gather desc prep here: after pass A — the no_sync fence keeps the
        # preps (and their DMASW-lane reset instructions) scheduled after the
        # scatters, so the pre-scatter reset wall is halved; still before any
        # Y write exists in program order (no Y waits on the preps).
        if run2 and run3:
            tc.no_sync_barrier()
        if run2 and run3:
            for q in range(NQ):
                nc.gpsimd.dma_gather(
                    yg[:, QN*q:QN*(q+1), :], Y[:],
                    idx16[:, (QI//16)*q:(QI//16)*(q+1)], QI, QI, RWH,
                    single_packet=True, prepare_only=True, sem=gsems[q],
                    queue_num=q)
        # pass B: FFN (ACT funcs: Silu only) + store
        for s in range(NSG) if run2 else []:
            yS = gpool.tile([P, PADSZ], f16, tag="yS")
            for i in range(4):
                psB = psp.tile([P, PADSZ], f32, tag="psA", bufs=2)
                nc.tensor.matmul(psB[:], ct["c_wB"][:, s, i, :],
                                 xn2all[:, s, :], start=True, stop=True)
                hS = gpool.tile([P, PADSZ], f16, tag="hS")
                if use_silu:
                    nc.scalar.activation(out=hS[:], in_=psB[:], func=ACTF.Silu,
                                         bias=ct["c_b1s"][:, s, i, :], scale=1.0)
                else:
                    hpre = gpool.tile([P, PADSZ], f32, tag="hpre")
                    nc.vector.tensor_scalar(out=hpre[:], in0=psB[:],
                                            scalar1=ct["c_b1s"][:, s, i, :],
                                            scalar2=None, op0=OP.add)
                    sg_ = gpool.tile([P, PADSZ], f32, tag="sg_")
                    nc.scalar.activation(out=sg_[:], in_=hpre[:],
                                         func=ACTF.Sigmoid, scale=1.0)
                    nc.vector.tensor_tensor(out=hS[:], in0=hpre[:], in1=sg_[:],
                                            op=OP.mult)
                psC = psp.tile([32, PADSZ], f32, tag="psC", bufs=2)
                nc.tensor.matmul(psC[:], ct["c_wC"][:, s, i, :], hS[:],
                                 start=True, stop=True)
                nc.vector.tensor_tensor(out=yS[32*i:32*(i+1), :],
                                        in0=x1pball[32*i:32*(i+1), s, :],
                                        in1=psC[:], op=OP.add)
            pto1 = psp.tile([P, P], f16, tag="ptt")
            nc.tensor.transpose(pto1[:], yS[:, 0:P], ct["c_id16"][:])
            yT1 = gpool.tile([P, P], f16, tag="yT1")
            nc.vector.tensor_copy(out=yT1[:], in_=pto1[:])
            nc.sync.dma_start(out=Yv[0:P, 8*s:8*s+8, 0:D],
                              in_=yT1[:].rearrange("c (e d) -> c e d", e=8))
            pto2 = psp.tile([32, P], f16, tag="ptt")
            nc.tensor.transpose(pto2[:], yS[:, P:PADSZ], ct["c_id16"][:])
            yT2 = gpool.tile([32, P], f16, tag="yT2")
            nc.vector.tensor_copy(out=yT2[:], in_=pto2[:])
            nc.sync.dma_start(out=Yv[P:PADSZ, 8*s:8*s+8, 0:D],
                              in_=yT2[:].rearrange("c (e d) -> c e d", e=8))

        if run2 and not run3:
            nc.sync.dma_start(out=out.rearrange("(p n) d -> p n d", p=P),
                              in_=xnst[:, :, 0:D])

        # ---- phase 3: trigger the prepped gathers once Y landed. The Y
        # dependency is made explicit: read back slices covering every Y write
        # (rows 0:40 hit the c=0 rows of all 5 sgs' yT1 writes, rows
        # 5120:5160 the c=128 rows of the yT2 writes), then consume them on
        # gpsimd so the triggers are engine-ordered behind landed data. ----
        if run3:
            if not EARLY_PREP:
                for q in range(NQ):
                    nc.gpsimd.dma_gather(
                        yg[:, QN*q:QN*(q+1), :], Y[:],
                        idx16[:, (QI//16)*q:(QI//16)*(q+1)], QI, QI, RWH,
                        single_packet=True, prepare_only=True, sem=gsems[q],
                        queue_num=q)
            tc.strict_bb_all_engine_barrier()
            for q in range(NQ):
                nc.gpsimd.trigger_dma(count=None, queue_num=q)
            for q in range(NQ):
                nc.sync.wait_ge(gsems[q], 16)
                nc.sync.dma_start(
                    out=out.rearrange("(p n) d -> p n d", p=P)[:, QN*q:QN*(q+1), :],
                    in_=yg[:, QN*q:QN*(q+1), 0:D])

    nc.finalize()
    return nc


_CACHE = {}


def _get_nc():
    if "nc" not in _CACHE:
        _CACHE["nc"] = build_kernel()
    return _CACHE["nc"]


def _make_in_maps(state, Wv, Wo, W1, b1, W2, b2, ncores=8):
    state = np.ascontiguousarray(np.asarray(state, dtype=np.float32))
    consts = prep_consts(np.asarray(Wv, np.float32), np.asarray(Wo, np.float32),
                         np.asarray(W1, np.float32), np.asarray(b1, np.float32),
                         np.asarray(W2, np.float32), np.asarray(b2, np.float32))
    in_maps = []
    for c in range(ncores):
        m = {"state": state[c * Bc:(c + 1) * Bc]}
        m.update(consts)
        in_maps.append(m)
    return in_maps


def kernel(state, Wq, Wk, Wv, Wo, W1, b1, W2, b2, **_unused):
    from concourse.bass_utils import run_bass_kernel_spmd

    nc = _get_nc()
    in_maps = _make_in_maps(state, Wv, Wo, W1, b1, W2, b2)
    res = run_bass_kernel_spmd(nc, in_maps, core_ids=list(range(8)))
    out = np.concatenate([res.results[c]["out"] for c in range(8)], axis=0)
    return out.astype(np.float32)


def _install_ntff_hook():
    """Inject the missing antenv.axon_hooks glue so trace=True works under axon."""
    import types
    if "antenv.axon_hooks" in sys.modules:
        return
    import antenv
    mod = types.ModuleType("antenv.axon_hooks")
    _state = {"hook": None}
    mod.set_axon_ntff_profile_hook = lambda h: _state.__setitem__("hook", h)
    mod.get_axon_ntff_profile_hook = lambda: _state["hook"]
    sys.modules["antenv.axon_hooks"] = mod
    antenv.axon_hooks = mod
    from trn_agent_boot.trn_boot import _ntff_profile_via_ctypes
    hook = _ntff_profile_via_ctypes("/opt/axon/libaxon_pjrt.so")
    if hook is not None:
        mod.set_axon_ntff_profile_hook(hook)


def profile_exec_time(inputs, tmpdir=None):
    """Run once with NTFF tracing; return max per-core HW exec time in ns."""
    _install_ntff_hook()
    import concourse.bass_utils as bu
    bu.upload_artifacts = lambda d: d  # zero-egress container: skip S3 upload

    nc = _get_nc()
    in_maps = _make_in_maps(inputs["state"], inputs["Wv"], inputs["Wo"],
                            inputs["W1"], inputs["b1"], inputs["W2"],
                            inputs["b2"])
    res = bu.run_bass_kernel_spmd(nc, in_maps, core_ids=list(range(8)),
                                  trace=True, tmpdir=tmpdir)
    return res.exec_time_ns


# revision 49
# speedup vs baseline: 1.0414x; 1.0414x over previous
"""Trainium2 Bass kernel for nn_C4StandardTransformer (MoE-routed transformer step).

kernel(**inputs) takes FULL inputs (state [32768,16] + expert weights), shards
the batch across 8 NeuronCores (pure data parallel), runs an on-device
MoE-routed Bass kernel per core, and returns the full [32768,16] output.

Design (v4):
 - attention softmax over length-1 axis => w == 1; attn = Wov @ xn,
   Wov = Wo @ Wv per expert; Wq/Wk dead.
 - opcode is integral => top-hat gate is g0 = sigmoid(10)^2 own-expert only
   (neighbors ~4.5e-5, dropped). g0 folded into constants.
 - counting-sort routing on DVE + one PE matmul (strict-upper prefix);
   scatter/gather desc-gen is Q7-bound (~6.7ns/idx), so both are split
   across the 4 SWDGE queues (cpu pair q runs queue q) for 4x parallel
   generation; dummy 16-idx scatters at t=0 pay the ~7us IRAM library
   load off the critical path.
 - gather descriptors are prepared right after the scatters (before any Y
   write exists in program order) so gen overlaps phase 2; explicit ysem
   makes the per-queue triggers wait for the Y writes to land.
 - p-major token labeling: SBUF st[p][n] = DRAM token p*32+n, so input and
   output DMAs are 128 x 2KB contiguous descriptors (not 4096 x 64B).
 - supergroups of 8 experts processed in an 8-token-stacked [128,160] fp16
   layout (residual fused into the attn matmul via lhsT = [g0*I | g0*Wov^T]).
 - scalar-engine LUT swaps cost 1.28us, so phase 2 is split into pass A
   (stats; Copy/Square only), one grouped Sqrt batch, and pass B (Silu only).
"""
import sys
import numpy as np

for _p in ("/opt/trn_rl_repo", "/root/.axon_site/_ro/trn_rl_repo"):
    if _p not in sys.path:
        sys.path.append(_p)

E, D, DFF, OPCODE, EPS = 39, 16, 64, 6, 1e-5
Bc = 4096            # tokens per core
P = 128              # partitions
NCOL = Bc // P       # 32 free-dim token slots per partition
PADSZ = 160          # rank slots per expert per core
NE = 40              # padded expert count (8*5)
NSG = 5              # supergroups
NROW = PADSZ * NE    # sorted buffer rows
RWH = 128            # sorted-buffer row width in fp16 (256B stride)
G0 = float(1.0 / (1.0 + np.exp(-10.0))) ** 2
NQ = 4               # SWDGE queues (4 Q7 core pairs)
QI = Bc // NQ        # idxs per queue (1024)
QN = NCOL // NQ      # token cols per queue (8)
WARMUP = 104          # PE keep-alive matmuls across the scatter window
EARLY_PREP = True    # issue gather desc prep before phase 2 (overlap gen)
PRELOAD = False       # dummy scatters at t=0 to preload the Q7 IRAM library


def prep_consts(Wv, Wo, W1, b1, W2, b2):
    """Host-side constant packing. Returns dict name -> np.ndarray."""
    Wov = np.einsum('ejv,evd->ejd', Wo, Wv).astype(np.float32)  # attn = Wov @ xn

    def wslot(Warr, e, fill_shape):
        if 0 <= e < E:
            return Warr[e]
        return np.zeros(fill_shape, Warr.dtype)

    consts = {}
    consts["c_iota"] = np.arange(E, dtype=np.float16).reshape(1, E)
    lt = np.tril(np.ones((NCOL, NCOL), np.float16), -1)  # mask[n, n'] = n' < n
    consts["c_ltmask"] = lt.reshape(1, NCOL, NCOL)
    consts["c_uones"] = np.triu(np.ones((P, P), np.float16), 1)
    consts["c_id16"] = np.eye(P, dtype=np.float16)
    onesbd = np.zeros((P, P), np.float16)
    for t in range(8):
        onesbd[t*16:(t+1)*16, t*16:(t+1)*16] = np.float16(1.0 / 16.0)
    consts["c_onesbd"] = onesbd
    esel = np.zeros((8, P, P), np.float16)
    for r in range(8):
        for m in range(P):
            esel[r, 16 * r + (m % 16), m] = 1.0
    consts["c_esel"] = np.ascontiguousarray(esel.transpose(1, 0, 2))  # [P,8,P]

    # fused x1 weights: psA[16j+d, c] = g0*(state_d + sum_v Wov[e][d,v] xn_v)
    wI = np.zeros((NSG, 2, P, 64), np.float16)
    b2s = np.zeros((NSG, P, 1), np.float32)
    wB = np.zeros((NSG, 4, P, P), np.float16)
    b1s = np.zeros((NSG, 4, P, 1), np.float32)
    wC = np.zeros((NSG, 4, P, 32), np.float16)
    for s in range(NSG):
        for t in range(8):
            e = 8 * s + t
            b2s[s, t*16:(t+1)*16, 0] = G0 * wslot(b2, e, (D,))
            h, j = t // 4, t % 4
            wv = wslot(Wov, e, (D, D))  # [d_out, v_in]
            for dd in range(D):
                wI[s, h, 32*j + dd, 16*j + dd] = np.float16(G0)
            wI[s, h, 32*j+16:32*j+32, 16*j:16*j+16] = \
                (G0 * wv.T).astype(np.float16)        # [v, d_out]
        for i in range(4):
            for tt in range(2):
                e = 8 * s + 2 * i + tt
                t = 2 * i + tt
                w1 = wslot(W1, e, (DFF, D))           # h[f] = sum_d w1[f,d] xn2[d]
                wB[s, i, t*16:(t+1)*16, tt*64:(tt+1)*64] = w1.T.astype(np.float16)
                b1s[s, i, tt*64:(tt+1)*64, 0] = wslot(b1, e, (DFF,))
                w2 = wslot(W2, e, (D, DFF))           # ffn[d] = sum_f w2[d,f] h[f]
                wC[s, i, tt*64:(tt+1)*64, tt*16:(tt+1)*16] = \
                    (G0 * w2.T).astype(np.float16)
    consts["c_wI"] = np.ascontiguousarray(wI.transpose(2, 0, 1, 3))      # [P,5,2,64]
    consts["c_b2s"] = np.ascontiguousarray(b2s.transpose(1, 0, 2))       # [P,5,1]
    consts["c_wB"] = np.ascontiguousarray(wB.transpose(2, 0, 1, 3))      # [P,5,4,128]
    consts["c_b1s"] = np.ascontiguousarray(b1s.transpose(2, 0, 1, 3))    # [P,5,4,1]
    consts["c_wC"] = np.ascontiguousarray(wC.transpose(2, 0, 1, 3))      # [P,5,4,32]
    return consts


def build_kernel(stop_after=None, debug=False, use_silu=True):
    """stop_after in (None, 'route', 'scatter', 'compute'). use_silu=False
    falls back to Sigmoid+mult (CoreSim has no Silu)."""
    import concourse.bass as bass
    import concourse.bacc as bacc
    import concourse.tile as tile
    from concourse import mybir
    from contextlib import ExitStack

    f32, f16, i16 = mybir.dt.float32, mybir.dt.float16, mybir.dt.int16
    AX = mybir.AxisListType.X
    OP = mybir.AluOpType
    ACTF = mybir.ActivationFunctionType

    nc = bacc.Bacc(None, target_bir_lowering=False, num_swdge_queues=NQ)

    state = nc.declare_dram_parameter("state", [Bc, D], f32, isOutput=False)
    out = nc.declare_dram_parameter("out", [Bc, D], f16, isOutput=True)

    cshape = {
        "c_iota": ([1, E], f16), "c_ltmask": ([1, NCOL, NCOL], f16),
        "c_uones": ([P, P], f16), "c_esel": ([P, 8, P], f16),
        "c_id16": ([P, P], f16), "c_onesbd": ([P, P], f16),
        "c_wI": ([P, NSG, 2, 64], f16), "c_b2s": ([P, NSG, 1], f32),
        "c_wB": ([P, NSG, 4, P], f16), "c_b1s": ([P, NSG, 4, 1], f32),
        "c_wC": ([P, NSG, 4, 32], f16),
    }
    cparams = {n: nc.declare_dram_parameter(n, list(sh), dt, isOutput=False)
               for n, (sh, dt) in cshape.items()}

    if debug:
        Y = nc.declare_dram_parameter("Y", [NE * 256, RWH], f16, isOutput=True)
    else:
        Y = nc.dram_tensor("Y", [NE * 256, RWH], f16)
    TRASH = nc.dram_tensor("TRASH", [16, RWH], f16)

    run2 = stop_after not in ("route", "scatter")
    run3 = run2 and stop_after != "compute"

    with tile.TileContext(nc) as tc, ExitStack() as ctx:
        cpool = ctx.enter_context(tc.tile_pool(name="consts", bufs=1))
        ppool = ctx.enter_context(tc.tile_pool(name="p1", bufs=1))
        gpool = ctx.enter_context(tc.tile_pool(name="p2", bufs=2))
        psp = ctx.enter_context(tc.tile_pool(name="psp", bufs=1, space="PSUM"))

        # ---- SWDGE ucode preload: 4 dummy 16-idx scatters (one per queue)
        # pay the ~7us extended-inst IRAM load at t=0, off the critical path,
        # and verify all 4 queue core-pairs are live. ----
        if PRELOAD:
            zidx = cpool.tile([P, 16], i16, tag="zidx")
            nc.vector.memset(zidx[:], 0)
            zpay = cpool.tile([P, 1, 2 * D], f16, tag="zpay")
            nc.vector.memset(zpay[:], 0.0)
            for q in range(NQ):
                nc.gpsimd.dma_scatter_add(
                    TRASH[:, 0:2*D], zpay[:], zidx[:, q:q+1], 16, 16, 2 * D,
                    elem_step=RWH, single_packet=False, queue_num=q)

        # ---- phase 1 input first (gates everything); p-major token order:
        # st[p][n] = DRAM token p*NCOL+n -> 128 contiguous 2KB descriptors ----
        st = ppool.tile([P, NCOL, D], f32, tag="st")
        nc.sync.dma_start(out=st[:], in_=state.rearrange("(p n) d -> p n d", p=P))

        # ---- constants into SBUF (alternate queues, overlap phase 1);
        # routing-critical consts go on sync so scalar is free for opvh ----
        # routing-critical consts go on scalar (issued after the ACT table
        # load), bulk weights follow the input load on sync — so the big st
        # DMA issues first and routing starts ~3us earlier.
        ct = {}
        _crit = {"c_iota", "c_ltmask", "c_uones", "c_esel", "c_id16"}
        for ci, (n, (sh, dt)) in enumerate(cshape.items()):
            eng = nc.scalar if n in _crit else nc.sync
            if sh[0] == 1:
                rsh = [P] + list(sh[1:])
                t = cpool.tile(rsh, dt, tag=n)
                eng.dma_start(out=t[:], in_=cparams[n][:].to_broadcast(rsh))
            else:
                t = cpool.tile(sh, dt, tag=n)
                eng.dma_start(out=t[:], in_=cparams[n][:])
            ct[n] = t
        epsb = cpool.tile([P, 1], f32, tag="epsb")
        nc.vector.memset(epsb[:], EPS)

        # ---- sorted-token SBUF accumulators (parity-split scatter dst):
        # bufA holds ranks 0-127 (partition = rank), bufB ranks 128-159 on
        # partitions 0-31; free dim = expert column * 32 payload halves.
        # CCE-add needs them zeroed; memsets are engine-synchronous so the
        # scatter (ordered after by Tile WAW) adds onto zeros. ----
        ssems = [nc.alloc_semaphore(f"ssem{q}") for q in range(NQ)]
        bufA = ppool.tile([P, NE, 2 * D], f16, tag="bufA")
        bufB = ppool.tile([P, NE, 2 * D], f16, tag="bufB")
        nc.vector.memset(bufA[:], 0.0)
        nc.vector.memset(bufB[:], 0.0)
        zb = cpool.tile([P, 512], f16, tag="zb")
        nc.vector.memset(zb[:], 0.0)

        # ---- routing ----
        opvh = ppool.tile([P, NCOL], f16, tag="opvh")
        nc.scalar.copy(out=opvh[:], in_=st[:, :, OPCODE])
        eq39T = ppool.tile([P, E, NCOL], f16, tag="eq39T")
        nc.vector.tensor_tensor(
            out=eq39T[:],
            in0=opvh[:].rearrange("p (u n) -> p u n", u=1).to_broadcast([P, E, NCOL]),
            in1=ct["c_iota"][:].rearrange("p (e u) -> p e u", u=1).to_broadcast([P, E, NCOL]),
            op=OP.is_equal)
        rowcnth = ppool.tile([P, E], f16, tag="rowcnth")
        with nc.allow_low_precision(reason="counts <= 160 are fp16-exact"):
            nc.vector.tensor_reduce(out=rowcnth[:], in_=eq39T[:], axis=AX,
                                    op=OP.add)
        pidx = psp.tile([P, 512], f32, tag="pidx")
        nc.tensor.matmul(pidx[:, 0:E], ct["c_uones"][:], rowcnth[:], start=True,
                         stop=True)
        C1h = ppool.tile([P, E], f16, tag="C1h")
        nc.scalar.copy(out=C1h[:], in_=pidx[:, 0:E])
        eq39 = ppool.tile([P, NCOL, E], f16, tag="eq39")
        nc.vector.tensor_tensor(
            out=eq39[:],
            in0=opvh[:].rearrange("p (n u) -> p n u", u=1).to_broadcast([P, NCOL, E]),
            in1=ct["c_iota"][:].rearrange("p (u e) -> p u e", u=1).to_broadcast([P, NCOL, E]),
            op=OP.is_equal)
        mselh = ppool.tile([P, NCOL, E], f16, tag="mselh")
        nc.vector.tensor_tensor(
            out=mselh[:], in0=eq39[:],
            in1=C1h[:].rearrange("p (u e) -> p u e", u=1).to_broadcast([P, NCOL, E]),
            op=OP.mult)
        C1sel = ppool.tile([P, NCOL], f16, tag="C1sel")
        with nc.allow_low_precision(reason="counts <= 160 are fp16-exact"):
            nc.vector.tensor_reduce(out=C1sel[:], in_=mselh[:], axis=AX,
                                    op=OP.add)
        eqp = ppool.tile([P, NCOL, NCOL], f16, tag="eqp")
        nc.vector.tensor_tensor(
            out=eqp[:],
            in0=opvh[:].rearrange("p (n u) -> p n u", u=1).to_broadcast([P, NCOL, NCOL]),
            in1=opvh[:].rearrange("p (u n) -> p u n", u=1).to_broadcast([P, NCOL, NCOL]),
            op=OP.is_equal)
        nc.vector.tensor_tensor(out=eqp[:], in0=eqp[:],
                                in1=ct["c_ltmask"][:], op=OP.mult)
        c2h = ppool.tile([P, NCOL], f16, tag="c2h")
        with nc.allow_low_precision(reason="counts <= 32 are fp16-exact"):
            nc.vector.tensor_reduce(out=c2h[:], in_=eqp[:], axis=AX, op=OP.add)
        rnk = ppool.tile([P, NCOL], f16, tag="rnk")
        nc.vector.tensor_tensor(out=rnk[:], in0=C1sel[:], in1=c2h[:], op=OP.add)

        # ---- idx relayout via PE: pidx[16k+q, 32r+n] = {rnk|opv}[16r+q, n]
        # (dst = 40*rank + e assembled below; replication across the 8
        # 16-partition stripes comes from the replicated selector) ----
        for r in range(8):
            nc.tensor.matmul(pidx[:, 32*r:32*(r+1)], ct["c_esel"][:, r, :],
                             rnk[:], start=True, stop=True)
            nc.tensor.matmul(pidx[:, 256+32*r:256+32*(r+1)],
                             ct["c_esel"][:, r, :], opvh[:], start=True,
                             stop=True)
        pe2c = ppool.tile([P, 256], f16, tag="pe2c")
        nc.scalar.copy(out=pe2c[:], in_=pidx[:, 256:512])
        t1 = ppool.tile([P, 256], f32, tag="t1")
        nc.vector.tensor_scalar(out=t1[:], in0=pe2c[:],
                                scalar1=256.0, scalar2=None, op0=OP.mult)
        nc.vector.tensor_tensor(out=t1[:], in0=t1[:], in1=pidx[:, 0:256],
                                op=OP.add)
        idx16 = ppool.tile([P, Bc // 16], i16, tag="idx16")
        nc.vector.tensor_copy(
            out=idx16[:].rearrange("p (n r) -> p n r", r=8),
            in_=t1[:].rearrange("p (r n) -> p n r", r=8))

        # ---- LN1 -> payload [state | xn] fp16 ----
        xnst = ppool.tile([P, NCOL, 2 * D], f16, tag="xnst")
        nc.scalar.copy(out=xnst[:, :, 0:D], in_=st[:])
        mh = ppool.tile([P, NCOL], f32, tag="mh")
        nc.vector.tensor_reduce(out=mh[:], in_=st[:], axis=AX, op=OP.add)
        nc.vector.tensor_scalar(out=mh[:], in0=mh[:], scalar1=1.0 / D,
                                scalar2=None, op0=OP.mult)
        sqt = ppool.tile([P, NCOL, D], f32, tag="sqt")
        nc.scalar.square(out=sqt[:], in_=st[:])
        qh = ppool.tile([P, NCOL], f32, tag="qh")
        nc.vector.tensor_reduce(out=qh[:], in_=sqt[:], axis=AX, op=OP.add)
        msq = ppool.tile([P, NCOL], f32, tag="msq")
        nc.vector.tensor_tensor(out=msq[:], in0=mh[:], in1=mh[:], op=OP.mult)
        nc.vector.tensor_scalar(out=qh[:], in0=qh[:], scalar1=1.0 / D,
                                scalar2=None, op0=OP.mult)
        nc.vector.tensor_tensor(out=qh[:], in0=qh[:], in1=msq[:], op=OP.subtract)
        sd = ppool.tile([P, NCOL], f32, tag="sd")
        nc.scalar.activation(out=sd[:], in_=qh[:], func=ACTF.Sqrt,
                             bias=epsb[:], scale=1.0)
        rs = ppool.tile([P, NCOL], f32, tag="rs")
        nc.vector.reciprocal_approx_fast(out=rs[:], in_=sd[:])
        xc = ppool.tile([P, NCOL, D], f32, tag="xc")
        nc.vector.tensor_tensor(
            out=xc[:], in0=st[:],
            in1=mh[:].rearrange("p (n u) -> p n u", u=1).to_broadcast([P, NCOL, D]),
            op=OP.subtract)
        nc.vector.tensor_tensor(
            out=xnst[:, :, D:2*D], in0=xc[:],
            in1=rs[:].rearrange("p (n u) -> p n u", u=1).to_broadcast([P, NCOL, D]),
            op=OP.mult)

        # ---- scatter, split across the 4 SWDGE queues: queue q moves token
        # cols [8q, 8q+8) (idx positions [1024q, 1024q+1024)); desc-gen runs
        # on Q7 core pair q, so all 4 generate concurrently. ----
        for q in range(NQ):
            nc.gpsimd.dma_scatter_add(
                bufA[:], xnst[:, QN*q:QN*(q+1), :],
                idx16[:, (QI//16)*q:(QI//16)*(q+1)], QI, QI, 2 * D,
                single_packet=True, queue_num=q,
                sbuf_tokens_per_rank=P, parity_reg=0,
                out_ap_other=bufB[:]).then_inc(ssems[q], 16)

        # ---- gather desc prep, also split across queues, issued BEFORE any
        # Y write exists in program order (so Tile attaches no Y-write wait);
        # core pair q runs it right after its scatter gen, hidden under the
        # scatter drain + phase 2. Triggered after ysem confirms Y landed. ----
        tc.no_sync_barrier()
        gsems = [nc.alloc_semaphore(f"gsem{q}") for q in range(NQ)]
        yg = ppool.tile([P, NCOL, RWH], f16, tag="yg")

        if not run2:
            nc.sync.dma_start(out=out.rearrange("(p n) d -> p n d", p=P),
                              in_=xnst[:, :, 0:D])

        # ---- PE warm-up chain: keeps the HAM clock at 2.4GHz through the
        # scatter window (PE otherwise idles and re-throttles).
        # WAW-chained on one psum bank; gated after the relayout reads. ----
        if run2:
            for _w in range(WARMUP):
                nc.tensor.matmul(pidx[:, 0:512], ct["c_id16"][:],
                                 zb[:, 0:512], start=True, stop=True)

        # ---- phase 2: supergroups ----
        Yv = Y.rearrange("(e c) w -> c e w", c=256)
        if run2:
            vvall = ppool.tile([P, NSG, PADSZ], f32, tag="vvall")
            x1call = ppool.tile([P, NSG, PADSZ], f16, tag="x1call")
            x1pball = ppool.tile([P, NSG, PADSZ], f32, tag="x1pball")
            sdall = ppool.tile([P, NSG, PADSZ], f32, tag="sdall")
            rsall = ppool.tile([P, NSG, PADSZ], f32, tag="rsall")
            xn2all = ppool.tile([P, NSG, PADSZ], f16, tag="xn2all")
        if run2:
            for q in range(NQ):
                nc.tensor.wait_ge(ssems[q], 16)
        # pass A: stats (ACT funcs: Copy/Square only)
        for s in range(NSG) if run2 else []:
            xh = [gpool.tile([P, PADSZ], f16, tag=f"xh{h}", name=f"xh{h}")
                  for h in range(2)]
            for h in range(2):
                eb = 8 * s + 4 * h
                pt = psp.tile([P, PADSZ], f16, tag="ptt")
                nc.tensor.transpose(
                    pt[:, 0:P], bufA[:, eb:eb+4, :].rearrange("p e v -> p (e v)"),
                    ct["c_id16"][:])
                nc.tensor.transpose(
                    pt[:, P:PADSZ],
                    bufB[0:32, eb:eb+4, :].rearrange("p e v -> p (e v)"),
                    ct["c_id16"][0:32, 0:32])
                nc.scalar.copy(out=xh[h][:], in_=pt[:])
            psA = psp.tile([P, PADSZ], f32, tag="psA", bufs=2)
            for h in range(2):
                nc.tensor.matmul(psA[64*h:64*(h+1), :], ct["c_wI"][:, s, h, :],
                                 xh[h][:], start=True, stop=True)
            x1sq = gpool.tile([P, 2 * PADSZ], f16, tag="x1sq")
            nc.vector.tensor_copy(out=x1sq[:, 0:PADSZ], in_=psA[:])
            nc.vector.tensor_tensor(out=x1sq[:, PADSZ:2*PADSZ],
                                    in0=x1sq[:, 0:PADSZ], in1=psA[:],
                                    op=OP.mult)
            psS = psp.tile([P, 2 * PADSZ], f32, tag="psS", bufs=2)
            nc.tensor.matmul(psS[:], ct["c_onesbd"][:], x1sq[:], start=True,
                             stop=True)
            mc = gpool.tile([P, PADSZ], f32, tag="mc")
            nc.scalar.copy(out=mc[:], in_=psS[:, 0:PADSZ])
            msq2 = gpool.tile([P, PADSZ], f32, tag="msq2")
            nc.vector.tensor_tensor(out=msq2[:], in0=mc[:], in1=mc[:], op=OP.mult)
            nc.vector.tensor_tensor(out=vvall[:, s, :],
                                    in0=psS[:, PADSZ:2*PADSZ],
                                    in1=msq2[:], op=OP.subtract)
            nc.vector.tensor_tensor(out=x1call[:, s, :], in0=psA[:], in1=mc[:],
                                    op=OP.subtract)
            nc.vector.tensor_scalar(out=x1pball[:, s, :], in0=psA[:],
                                    scalar1=ct["c_b2s"][:, s, :],
                                    scalar2=None, op0=OP.add)
            # per-sg LN2 tail: Copy/Square are tableless, so the Sqrt table
            # stays resident through all of pass A (one load total).
            nc.scalar.activation(out=sdall[:, s, :], in_=vvall[:, s, :],
                                 func=ACTF.Sqrt, bias=epsb[:], scale=1.0)
            nc.vector.reciprocal_approx_fast(out=rsall[:, s, :],
                                             in_=sdall[:, s, :])
            nc.vector.tensor_tensor(out=xn2all[:, s, :],
                                    in0=x1call[:, s, :],
                                    in1=rsall[:, s, :], op=OP.mult)
        # gather desc prep here: after pass A — the no_sync fence keeps the
        # preps (and their DMASW-lane reset instructions) scheduled after the
        # scatters, so the pre-scatter reset wall is halved; still before any
        # Y write exists in program order (no Y waits on the preps).
        if run2 and run3:
            tc.no_sync_barrier()
        if run2 and run3:
            for q in range(NQ):
                nc.gpsimd.dma_gather(
                    yg[:, QN*q:QN*(q+1), :], Y[:],
                    idx16[:, (QI//16)*q:(QI//16)*(q+1)], QI, QI, RWH,
                    single_packet=True, prepare_only=True, sem=gsems[q],
                    queue_num=q)
        # pass B: FFN (ACT funcs: Silu only) + store
        for s in range(NSG) if run2 else []:
            yS = gpool.tile([P, PADSZ], f16, tag="yS")
            for i in range(4):
                psB = psp.tile([P, PADSZ], f32, tag="psA", bufs=2)
                nc.tensor.matmul(psB[:], ct["c_wB"][:, s, i, :],
                                 xn2all[:, s, :], start=True, stop=True)
                hS = gpool.tile([P, PADSZ], f16, tag="hS")
                if use_silu:
                    nc.scalar.activation(out=hS[:], in_=psB[:], func=ACTF.Silu,
                                         bias=ct["c_b1s"][:, s, i, :], scale=1.0)
                else:
                    hpre = gpool.tile([P, PADSZ], f32, tag="hpre")
                    nc.vector.tensor_scalar(out=hpre[:], in0=psB[:],
                                            scalar1=ct["c_b1s"][:, s, i, :],
                                            scalar2=None, op0=OP.add)
                    sg_ = gpool.tile([P, PADSZ], f32, tag="sg_")
                    nc.scalar.activation(out=sg_[:], in_=hpre[:],
                                         func=ACTF.Sigmoid, scale=1.0)
                    nc.vector.tensor_tensor(out=hS[:], in0=hpre[:], in1=sg_[:],
                                            op=OP.mult)
                psC = psp.tile([32, PADSZ], f32, tag="psC", bufs=2)
                nc.tensor.matmul(psC[:], ct["c_wC"][:, s, i, :], hS[:],
                                 start=True, stop=True)
                nc.vector.tensor_tensor(out=yS[32*i:32*(i+1), :],
                                        in0=x1pball[32*i:32*(i+1), s, :],
                                        in1=psC[:], op=OP.add)
            pto1 = psp.tile([P, P], f16, tag="ptt")
            nc.tensor.transpose(pto1[:], yS[:, 0:P], ct["c_id16"][:])
            yT1 = gpool.tile([P, P], f16, tag="yT1")
            nc.vector.tensor_copy(out=yT1[:], in_=pto1[:])
            nc.sync.dma_start(out=Yv[0:P, 8*s:8*s+8, 0:D],
                              in_=yT1[:].rearrange("c (e d) -> c e d", e=8))
            pto2 = psp.tile([32, P], f16, tag="ptt")
            nc.tensor.transpose(pto2[:], yS[:, P:PADSZ], ct["c_id16"][:])
            yT2 = gpool.tile([32, P], f16, tag="yT2")
            nc.vector.tensor_copy(out=yT2[:], in_=pto2[:])
            nc.sync.dma_start(out=Yv[P:PADSZ, 8*s:8*s+8, 0:D],
                              in_=yT2[:].rearrange("c (e d) -> c e d", e=8))

        if run2 and not run3:
            nc.sync.dma_start(out=out.rearrange("(p n) d -> p n d", p=P),
                              in_=xnst[:, :, 0:D])

        # ---- phase 3: trigger the prepped gathers once Y landed. The Y
        # dependency is made explicit: read back slices covering every Y write
        # (rows 0:40 hit the c=0 rows of all 5 sgs' yT1 writes, rows
        # 5120:5160 the c=128 rows of the yT2 writes), then consume them on
        # gpsimd so the triggers are engine-ordered behind landed data. ----
        if run3:
            if not EARLY_PREP:
                for q in range(NQ):
                    nc.gpsimd.dma_gather(
                        yg[:, QN*q:QN*(q+1), :], Y[:],
                        idx16[:, (QI//16)*q:(QI//16)*(q+1)], QI, QI, RWH,
                        single_packet=True, prepare_only=True, sem=gsems[q],
                        queue_num=q)
            tc.strict_bb_all_engine_barrier()
            for q in range(NQ):
                nc.gpsimd.trigger_dma(count=None, queue_num=q)
            for q in range(NQ):
                nc.sync.wait_ge(gsems[q], 16)
                nc.sync.dma_start(
                    out=out.rearrange("(p n) d -> p n d", p=P)[:, QN*q:QN*(q+1), :],
                    in_=yg[:, QN*q:QN*(q+1), 0:D])

    nc.finalize()
    return nc


_CACHE = {}


def _get_nc():
    if "nc" not in _CACHE:
        _CACHE["nc"] = build_kernel()
    return _CACHE["nc"]


def _make_in_maps(state, Wv, Wo, W1, b1, W2, b2, ncores=8):
    state = np.ascontiguousarray(np.asarray(state, dtype=np.float32))
    consts = prep_consts(np.asarray(Wv, np.float32), np.asarray(Wo, np.float32),
                         np.asarray(W1, np.float32), np.asarray(b1, np.float32),
                         np.asarray(W2, np.float32), np.asarray(b2, np.float32))
    in_maps = []
    for c in range(ncores):
        m = {"state": state[c * Bc:(c + 1) * Bc]}
        m.update(consts)
        in_maps.append(m)
    return in_maps


def kernel(state, Wq, Wk, Wv, Wo, W1, b1, W2, b2, **_unused):
    from concourse.bass_utils import run_bass_kernel_spmd

    nc = _get_nc()
    in_maps = _make_in_maps(state, Wv, Wo, W1, b1, W2, b2)
    res = run_bass_kernel_spmd(nc, in_maps, core_ids=list(range(8)))
    out = np.concatenate([res.results[c]["out"] for c in range(8)], axis=0)
    return out.astype(np.float32)


def _install_ntff_hook():
    """Inject the missing antenv.axon_hooks glue so trace=True works under axon."""
    import types
    if "antenv.axon_hooks" in sys.modules:
        return
    import antenv
    mod = types.ModuleType("antenv.axon_hooks")
    _state = {"hook": None}
    mod.set_axon_ntff_profile_hook = lambda h: _state.__setitem__("hook", h)
    mod.get_axon_ntff_profile_hook = lambda: _state["hook"]
    sys.modules["antenv.axon_hooks"] = mod
    antenv.axon_hooks = mod
    from trn_agent_boot.trn_boot import _ntff_profile_via_ctypes
    hook = _ntff_profile_via_ctypes("/opt/axon/libaxon_pjrt.so")
    if hook is not None:
        mod.set_axon_ntff_profile_hook(hook)


def profile_exec_time(inputs, tmpdir=None):
    """Run once with NTFF tracing; return max per-core HW exec time in ns."""
    _install_ntff_hook()
    import concourse.bass_utils as bu
    bu.upload_artifacts = lambda d: d  # zero-egress container: skip S3 upload

    nc = _get_nc()
    in_maps = _make_in_maps(inputs["state"], inputs["Wv"], inputs["Wo"],
                            inputs["W1"], inputs["b1"], inputs["W2"],
                            inputs["b2"])
    res = bu.run_bass_kernel_spmd(nc, in_maps, core_ids=list(range(8)),
                                  trace=True, tmpdir=tmpdir)
    return res.exec_time_ns
